# revision 1
# baseline (speedup 1.0000x reference)
"""EntityEncoder forward kernel for 8 Trainium2 NeuronCores.

Reference computation (per entity row [unused, feat_a, azimuth, feat_b, type_id]):
    out[0]     = feat_a
    out[1]     = |az| / 180
    out[2]     = where(az >= -90, |90 - az|, 90 + |az + 180|) / 180
    out[3]     = feat_b
    out[4:128] = type_emb[int(type_id)]          # 124-wide embedding row

Sharding: data-parallel over the batch dim. Each of the 8 cores gets 32 of the
256 batches (65536 entities), the full embedding table, and produces its
65536x128 slice of the output. No collectives (forward only).

Per-core kernel layout ("p-major"): a tile covers 128*G entities; entity
e = tile*128*G + p*G + g lives on partition p, column-group g, so entity loads
and output stores are contiguous per partition.

The embedding fetch is `dma_gather` (one 512-byte padded table row per entity,
HBM -> SBUF). Its fixed contract is: index-list position j writes output
[j % 128, j // 128], and the int16 index list lives wrapped in 16 partitions
(position j at [j % 16, j // 16]) replicated across the 8 GpSimd cores.
The wrapped index tensor is precomputed on the host during sharding (a pure
relayout of entity channel 4); the table is zero-padded host-side to
[NTYPES, 128] so gathered rows land exactly on output rows, whose first 4
channels the DVE/ACT engines then overwrite in SBUF before the store.
"""

import os
import sys

import numpy as np

for _p in ("/opt/trn_rl_repo", "/root/.axon_site/_ro/trn_rl_repo"):
    if os.path.isdir(_p) and _p not in sys.path:
        sys.path.append(_p)

B, N, EMB, NTYPES = 256, 2048, 128, 1000
NCORES = 8
BP = B // NCORES            # batches per core
E_CORE = BP * N             # entities per core (65536)
G = 64                      # entities per partition per tile
TILE_E = 128 * G            # entities per tile (8192)
NT = E_CORE // TILE_E       # tiles per core (8)
WCOLS = TILE_E // 16        # wrapped index columns per tile (512)

INV180 = float(np.float32(1.0) / np.float32(180.0))


def build_nc(nt_run=NT, repeats=1, parts=("load", "gather", "compute", "store"),
             got_bufs=4, gather_queues=1, gather_splits=4):
    import concourse.bacc as bacc
    import concourse.mybir as mybir
    import concourse.tile as tile

    f32 = mybir.dt.float32
    i16 = mybir.dt.int16
    Alu = mybir.AluOpType
    Act = mybir.ActivationFunctionType

    nc = bacc.Bacc()
    ents = nc.declare_dram_parameter("entities", [NT, 128, G, 5], f32, isOutput=False)
    table = nc.declare_dram_parameter("type_emb_pad", [NTYPES, EMB], f32, isOutput=False)
    idxw = nc.declare_dram_parameter("idx_wrapped", [NT, 128, WCOLS], i16, isOutput=False)
    out = nc.declare_dram_parameter("out", [NT, 128, G, EMB], f32, isOutput=True)

    with tile.TileContext(nc) as tc:
        with (
            tc.tile_pool(name="entp", bufs=NT) as entp,
            tc.tile_pool(name="gotp", bufs=got_bufs) as gotp,
            tc.tile_pool(name="smallp", bufs=NT) as smallp,
            tc.tile_pool(name="constp", bufs=1) as constp,
        ):
            neg_half = constp.tile([128, 1], f32, tag="neg_half")
            nc.vector.memset(neg_half[:], -0.5)
            for t in [t for _ in range(repeats) for t in range(nt_run)]:
                if "load" in parts:
                    ent = entp.tile([128, G, 5], f32, tag="ent")
                    nc.sync.dma_start(out=ent[:], in_=ents[t])
                idx = smallp.tile([128, WCOLS], i16, tag="idx")
                nc.sync.dma_start(out=idx[:], in_=idxw[t])

                got = gotp.tile([128, G, EMB], f32, tag="got")
                if "gather" in parts:
                    # Gather position j fills got[j%128, j//128]; a split at
                    # j0 = TILE_E*s/splits is a clean split of the idx list
                    # (cols j0//16:) and of got's column groups (g j0//128:).
                    ns = gather_splits
                    for s in range(ns):
                        nc.gpsimd.dma_gather(
                            out_ap=got[:, s * (G // ns):(s + 1) * (G // ns), :],
                            in_ap=table[:, :],
                            idxs_ap=idx[:, s * (WCOLS // ns):(s + 1) * (WCOLS // ns)],
                            num_idxs=TILE_E // ns,
                            num_idxs_reg=TILE_E // ns,
                            elem_size=EMB,
                            single_packet=False,
                            queue_num=t % gather_queues,
                        )

                if "compute" in parts:
                    az = ent[:, :, 2]
                    # out0 = feat_a ; out3 = feat_b
                    nc.vector.tensor_copy(out=got[:, :, 0], in_=ent[:, :, 1])
                    nc.vector.tensor_copy(out=got[:, :, 3], in_=ent[:, :, 3])
                    # out1 = |az| / 180
                    nc.scalar.activation(got[:, :, 1], az, Act.Abs, bias=0.0, scale=INV180)
                    # out2 = where(az >= -90, |90-az|, 90+|az+180|) / 180
                    #      = min(|az/180 - 0.5|, |az/180 + 1| + 0.5)  (identical piecewise-linear W)
                    t1 = smallp.tile([128, G], f32, tag="t1")
                    t2 = smallp.tile([128, G], f32, tag="t2")
                    nc.scalar.activation(t1[:], az, Act.Abs, bias=neg_half[:], scale=INV180)
                    nc.scalar.activation(t2[:], az, Act.Abs, bias=1.0, scale=INV180)
                    nc.vector.tensor_scalar(t2[:], t2[:], 0.5, None, Alu.add)
                    nc.vector.tensor_tensor(out=got[:, :, 2], in0=t1[:], in1=t2[:], op=Alu.min)

                if "store" in parts:
                    nc.sync.dma_start(out=out[t], in_=got[:])
    nc.compile()
    return nc


def shard_inputs(entities, type_emb):
    """Full inputs -> per-core in_maps (p-major entity layout per tile)."""
    entities = np.ascontiguousarray(entities, dtype=np.float32)
    type_emb = np.ascontiguousarray(type_emb, dtype=np.float32)

    # Zero-pad table rows to 512B so a gathered row is exactly one output row.
    table_pad = np.zeros((NTYPES, EMB), dtype=np.float32)
    table_pad[:, 4:] = type_emb

    ents = entities.reshape(NCORES, NT, 128, G, 5)

    # Wrapped gather indices: position j of tile t must hold the type id of
    # the entity at out[j%128, j//128] (= p-major entity p*G + g with
    # p = j%128, g = j//128), stored at [j%16, j//16], replicated x8.
    ids = ents[:, :, :, :, 4].astype(np.int16)          # [NCORES, NT, 128(p), G]
    pos = ids.transpose(0, 1, 3, 2)                     # v[j] with j = g*128 + p
    pos = pos.reshape(NCORES, NT, TILE_E)               # index value at position j
    wrapped = pos.reshape(NCORES, NT, WCOLS, 16).transpose(0, 1, 3, 2)  # [.., 16, WCOLS]
    wrapped = np.ascontiguousarray(np.tile(wrapped, (1, 1, 8, 1)))      # [.., 128, WCOLS]

    return [
        {"entities": ents[c], "type_emb_pad": table_pad, "idx_wrapped": wrapped[c]}
        for c in range(NCORES)
    ]


def unshard_output(results):
    """Per-core result dicts -> full [B, N, EMB] output."""
    outs = [np.asarray(r["out"]).reshape(BP, N, EMB) for r in results]
    return np.concatenate(outs, axis=0)


def _spot_check(out, entities, type_emb, n=256):
    """Cheap host-side sanity check on a random sample of entities: the
    embedding slots must match the table row, slots 0/3 the input channels."""
    rng = np.random.default_rng(1234)
    ef = entities.reshape(-1, 5)
    of = out.reshape(-1, EMB)
    sel = rng.integers(0, ef.shape[0], n)
    ids = ef[sel, 4].astype(np.int64)
    if not np.array_equal(of[sel, 4:], type_emb[ids]):
        return False
    if not np.array_equal(of[sel, 0], ef[sel, 1]):
        return False
    return bool(np.isfinite(of[sel]).all())


def kernel(entities, type_emb):
    from concourse.bass_utils import run_bass_kernel_spmd

    entities = np.ascontiguousarray(entities, dtype=np.float32)
    type_emb = np.ascontiguousarray(type_emb, dtype=np.float32)
    nc = build_nc()
    in_maps = shard_inputs(entities, type_emb)
    last_err = None
    for _attempt in range(3):
        try:
            res = run_bass_kernel_spmd(nc, in_maps, list(range(NCORES)))
            out = unshard_output(res.results)
            if _spot_check(out, entities, type_emb):
                return out
            last_err = RuntimeError("kernel output failed spot check")
        except Exception as e:  # flaky device sessions: retry
            last_err = e
    raise last_err



# revision 4
# speedup vs baseline: 2.9719x; 2.9719x over previous
"""EntityEncoder forward kernel for 8 Trainium2 NeuronCores.

Reference computation (per entity row [unused, feat_a, azimuth, feat_b, type_id]):
    out[0]     = feat_a
    out[1]     = |az| / 180
    out[2]     = where(az >= -90, |90 - az|, 90 + |az + 180|) / 180
    out[3]     = feat_b
    out[4:128] = type_emb[int(type_id)]          # 124-wide embedding row

Sharding: data-parallel over the batch dim. Each of the 8 cores gets 32 of the
256 batches (65536 entities), the full embedding table, and produces its
65536x128 slice of the output. No collectives (forward only).

Per-core kernel layout ("p-major"): a tile covers 128*G entities; entity
e = tile*128*G + p*G + g lives on partition p, column-group g, so entity loads
and output stores are contiguous per partition.

The embedding fetch is `dma_gather` (one 512-byte padded table row per entity,
HBM -> SBUF). Its fixed contract is: index-list position j writes output
[j % 128, j // 128], and the int16 index list lives wrapped in 16 partitions
(position j at [j % 16, j // 16]) replicated across the 8 GpSimd cores.
The wrapped index tensor is precomputed on the host during sharding (a pure
relayout of entity channel 4); the table is zero-padded host-side to
[NTYPES, 128] so gathered rows land exactly on output rows, whose first 4
channels the DVE/ACT engines then overwrite in SBUF before the store.
"""

import os
import sys

import numpy as np

for _p in ("/opt/trn_rl_repo", "/root/.axon_site/_ro/trn_rl_repo"):
    if os.path.isdir(_p) and _p not in sys.path:
        sys.path.append(_p)

B, N, EMB, NTYPES = 256, 2048, 128, 1000
NCORES = 8
BP = B // NCORES            # batches per core
E_CORE = BP * N             # entities per core (65536)
G = 64                      # entities per partition per tile
TILE_E = 128 * G            # entities per tile (8192)
NT = E_CORE // TILE_E       # tiles per core (8)
WCOLS = TILE_E // 16        # wrapped index columns per tile (512)

INV180 = float(np.float32(1.0) / np.float32(180.0))


def build_nc(nt_run=NT, repeats=1, parts=("load", "gather", "compute", "store"),
             got_bufs=4, gather_queues=1, gather_splits=4, single_packet=False):
    import concourse.bacc as bacc
    import concourse.mybir as mybir
    import concourse.tile as tile

    f32 = mybir.dt.float32
    i16 = mybir.dt.int16
    Alu = mybir.AluOpType
    Act = mybir.ActivationFunctionType

    nc = bacc.Bacc()
    ents = nc.declare_dram_parameter("entities", [NT, 128, G, 5], f32, isOutput=False)
    table = nc.declare_dram_parameter("type_emb_pad", [NTYPES, EMB], f32, isOutput=False)
    idxw = nc.declare_dram_parameter("idx_wrapped", [NT, 128, WCOLS], i16, isOutput=False)
    if "seqread" in parts:
        seqsrc = nc.declare_dram_parameter("seqread_src", [128, G, EMB], f32,
                                           isOutput=False)
    out = nc.declare_dram_parameter("out", [NT, 128, G, EMB], f32, isOutput=True)

    with tile.TileContext(nc) as tc:
        with (
            tc.tile_pool(name="entp", bufs=NT) as entp,
            tc.tile_pool(name="gotp", bufs=got_bufs) as gotp,
            tc.tile_pool(name="smallp", bufs=NT) as smallp,
            tc.tile_pool(name="constp", bufs=1) as constp,
        ):
            neg_half = constp.tile([128, 1], f32, tag="neg_half")
            nc.vector.memset(neg_half[:], -0.5)
            for t in [t for _ in range(repeats) for t in range(nt_run)]:
                if "load" in parts:
                    ent = entp.tile([128, G, 5], f32, tag="ent")
                    nc.sync.dma_start(out=ent[:], in_=ents[t])
                idx = smallp.tile([128, WCOLS], i16, tag="idx")
                nc.sync.dma_start(out=idx[:], in_=idxw[t])

                got = gotp.tile([128, G, EMB], f32, tag="got")
                if "seqread" in parts:
                    # 4MB/tile sequential HBM read (same bytes as the gather)
                    # via plain sync-engine DMA.
                    nc.sync.dma_start(out=got[:], in_=seqsrc[:])
                if "store" in parts and "gather" not in parts and "compute" not in parts \
                        and "seqread" not in parts:
                    nc.vector.memset(got[:, :, 0], 0.0)  # allocate tile for store-only
                if "gather" in parts:
                    # Gather position j fills got[j%128, j//128]; a split at
                    # j0 = TILE_E*s/splits is a clean split of the idx list
                    # (cols j0//16:) and of got's column groups (g j0//128:).
                    ns = gather_splits
                    for s in range(ns):
                        nc.gpsimd.dma_gather(
                            out_ap=got[:, s * (G // ns):(s + 1) * (G // ns), :],
                            in_ap=table[:, :],
                            idxs_ap=idx[:, s * (WCOLS // ns):(s + 1) * (WCOLS // ns)],
                            num_idxs=TILE_E // ns,
                            num_idxs_reg=TILE_E // ns,
                            elem_size=EMB,
                            single_packet=single_packet,
                            queue_num=t % gather_queues,
                        )

                if "compute" in parts:
                    az = ent[:, :, 2]
                    # out0 = feat_a ; out3 = feat_b
                    nc.vector.tensor_copy(out=got[:, :, 0], in_=ent[:, :, 1])
                    nc.vector.tensor_copy(out=got[:, :, 3], in_=ent[:, :, 3])
                    # out1 = |az| / 180
                    nc.scalar.activation(got[:, :, 1], az, Act.Abs, bias=0.0, scale=INV180)
                    # out2 = where(az >= -90, |90-az|, 90+|az+180|) / 180
                    #      = min(|az/180 - 0.5|, |az/180 + 1| + 0.5)  (identical piecewise-linear W)
                    t1 = smallp.tile([128, G], f32, tag="t1")
                    t2 = smallp.tile([128, G], f32, tag="t2")
                    nc.scalar.activation(t1[:], az, Act.Abs, bias=neg_half[:], scale=INV180)
                    nc.scalar.activation(t2[:], az, Act.Abs, bias=1.0, scale=INV180)
                    nc.vector.tensor_scalar(t2[:], t2[:], 0.5, None, Alu.add)
                    nc.vector.tensor_tensor(out=got[:, :, 2], in0=t1[:], in1=t2[:], op=Alu.min)

                if "store" in parts:
                    nc.sync.dma_start(out=out[t], in_=got[:])
    nc.compile()
    return nc


def shard_inputs(entities, type_emb):
    """Full inputs -> per-core in_maps (p-major entity layout per tile)."""
    entities = np.ascontiguousarray(entities, dtype=np.float32)
    type_emb = np.ascontiguousarray(type_emb, dtype=np.float32)

    # Zero-pad table rows to 512B so a gathered row is exactly one output row.
    table_pad = np.zeros((NTYPES, EMB), dtype=np.float32)
    table_pad[:, 4:] = type_emb

    ents = entities.reshape(NCORES, NT, 128, G, 5)

    # Wrapped gather indices: position j of tile t must hold the type id of
    # the entity at out[j%128, j//128] (= p-major entity p*G + g with
    # p = j%128, g = j//128), stored at [j%16, j//16], replicated x8.
    ids = ents[:, :, :, :, 4].astype(np.int16)          # [NCORES, NT, 128(p), G]
    pos = ids.transpose(0, 1, 3, 2)                     # v[j] with j = g*128 + p
    pos = pos.reshape(NCORES, NT, TILE_E)               # index value at position j
    wrapped = pos.reshape(NCORES, NT, WCOLS, 16).transpose(0, 1, 3, 2)  # [.., 16, WCOLS]
    wrapped = np.ascontiguousarray(np.tile(wrapped, (1, 1, 8, 1)))      # [.., 128, WCOLS]

    return [
        {"entities": ents[c], "type_emb_pad": table_pad, "idx_wrapped": wrapped[c]}
        for c in range(NCORES)
    ]


def unshard_output(results):
    """Per-core result dicts -> full [B, N, EMB] output."""
    outs = [np.asarray(r["out"]).reshape(BP, N, EMB) for r in results]
    return np.concatenate(outs, axis=0)


def _spot_check(out, entities, type_emb, n=256):
    """Cheap host-side sanity check on a random sample of entities: the
    embedding slots must match the table row, slots 0/3 the input channels."""
    rng = np.random.default_rng(1234)
    ef = entities.reshape(-1, 5)
    of = out.reshape(-1, EMB)
    sel = rng.integers(0, ef.shape[0], n)
    ids = ef[sel, 4].astype(np.int64)
    if not np.array_equal(of[sel, 4:], type_emb[ids]):
        return False
    if not np.array_equal(of[sel, 0], ef[sel, 1]):
        return False
    return bool(np.isfinite(of[sel]).all())


def kernel(entities, type_emb):
    from concourse.bass_utils import run_bass_kernel_spmd

    entities = np.ascontiguousarray(entities, dtype=np.float32)
    type_emb = np.ascontiguousarray(type_emb, dtype=np.float32)
    nc = build_nc()
    in_maps = shard_inputs(entities, type_emb)
    last_err = None
    for _attempt in range(3):
        try:
            res = run_bass_kernel_spmd(nc, in_maps, list(range(NCORES)))
            out = unshard_output(res.results)
            if _spot_check(out, entities, type_emb):
                return out
            last_err = RuntimeError("kernel output failed spot check")
        except Exception as e:  # flaky device sessions: retry
            last_err = e
    raise last_err

